# revision 1
# baseline (speedup 1.0000x reference)
"""Vocab-parallel sqrt-length-weighted cross-entropy loss on 8 NeuronCores.

Strategy (8-way vocab parallel):
  - proj_weight's vocab dim is sharded 8 ways (4000 rows/core). Each core
    computes partial logits for all 8192 tokens against its shard with a
    bf16 PE matmul, fusing exp+row-sum on the Scalar engine at PSUM
    eviction (no max subtraction needed: logits ~ N(0,1), exp is safe in
    f32). Local sum-exp vectors are AllReduce-summed across cores.
  - The target logit for each token is computed exactly in f32 as a
    DVE dot product of the token's activation with its gathered target
    weight row (host gathers the rows; each core handles a 1024-token
    slab) and AllGathered.
  - Every core then computes the final weighted reduction identically.
"""

import numpy as np
import ml_dtypes

B, S, D, V = 2, 4096, 1024, 32000
N_CORES = 8
P = 128
T = B * S                # 8192 tokens
KT = D // P              # 8 contraction tiles
N_TI = T // P            # 64 token tiles
V_SHARD = V // N_CORES   # 4000
VC = 500                 # vocab cols per matmul (one PSUM bank: 500*4B <= 2KB)
N_VC = V_SHARD // VC     # 8
T_SLAB = T // N_CORES    # 1024
SLAB_TI = T_SLAB // P    # 8
IGNORE = -100
EPS = 1e-8
SB_FREE = T // 2         # batch boundary in free dim of [128, 64] token layout
W_SCALE = 32.0           # pre-scale for fp8 weights (w ~ N(0, 1/1024))

_CACHE = {}


def _build():
    if "nc" in _CACHE:
        return _CACHE["nc"]

    from contextlib import ExitStack

    import concourse.bacc as bacc
    import concourse.mybir as mybir
    import concourse.tile as tile

    f32 = mybir.dt.float32
    f8 = mybir.dt.float8e4
    i32 = mybir.dt.int32
    Alu = mybir.AluOpType
    Act = mybir.ActivationFunctionType
    AX = mybir.AxisListType.X
    DR = mybir.MatmulPerfMode.DoubleRow

    nc = bacc.Bacc("TRN2", target_bir_lowering=False, debug=False,
                   num_devices=N_CORES)

    # Inputs (per core): pre-tiled on host.
    # xT[ti, p, k, t] = outputs_flat[ti*128 + t, k*128 + p]  (fp8 e4m3)
    xT = nc.dram_tensor("xT", [N_TI, P, KT, P], f8, kind="ExternalInput")
    # wT[p, k, v] = W_SCALE * w_shard[v, k*128 + p]  (fp8 e4m3)
    wT = nc.dram_tensor("wT", [P, KT, V_SHARD], f8, kind="ExternalInput")
    # gathered target weight rows + activations for this core's token slab
    w_tgt = nc.dram_tensor("w_tgt", [T_SLAB, D], f32, kind="ExternalInput")
    x_slab = nc.dram_tensor("x_slab", [T_SLAB, D], f32, kind="ExternalInput")
    tgts = nc.dram_tensor("tgts", [T], i32, kind="ExternalInput")
    loss_out = nc.dram_tensor("loss", [1, 1], f32, kind="ExternalOutput")

    # Collective bounce buffers (flat index = token id)
    se_in = nc.dram_tensor("se_in", [N_TI, P], f32)
    se_out = nc.dram_tensor("se_out", [N_TI, P], f32, addr_space="Shared")
    d_in = nc.dram_tensor("d_in", [SLAB_TI, P], f32)
    d_out = nc.dram_tensor("d_out", [N_TI, P], f32, addr_space="Shared")

    rg = [list(range(N_CORES))]

    with tile.TileContext(nc) as tc, ExitStack() as ctx:
        const = ctx.enter_context(tc.tile_pool(name="const", bufs=1))
        wpool = ctx.enter_context(tc.tile_pool(name="wpool", bufs=1))
        xpool = ctx.enter_context(tc.tile_pool(name="xpool", bufs=3))
        psum = ctx.enter_context(
            tc.tile_pool(name="psum", bufs=7, space="PSUM"))
        psfin = ctx.enter_context(
            tc.tile_pool(name="psfin", bufs=1, space="PSUM"))
        epool = ctx.enter_context(tc.tile_pool(name="epool", bufs=3))
        apool = ctx.enter_context(tc.tile_pool(name="apool", bufs=3))
        dpool = ctx.enter_context(tc.tile_pool(name="dpool", bufs=3))
        spool = ctx.enter_context(tc.tile_pool(name="spool", bufs=1))
        fin = ctx.enter_context(tc.tile_pool(name="fin", bufs=1))

        zero_b = const.tile([P, 1], f32)
        nc.vector.memset(zero_b[:], 0.0)
        eps_b = const.tile([P, 1], f32)
        nc.vector.memset(eps_b[:], EPS)
        ones = const.tile([P, 1], f32)
        nc.vector.memset(ones[:], 1.0)

        # Resident weight shard
        w_sb = wpool.tile([P, KT, V_SHARD], f8)
        nc.sync.dma_start(w_sb[:], wT[:])

        S_sb = spool.tile([P, N_TI], f32)    # per-token local sum-exp
        D_sb = spool.tile([P, SLAB_TI], f32)  # per-token target dot (slab)

        # ---- target-logit dots (DVE, f32 exact) ----
        for si in range(SLAB_TI):
            a = dpool.tile([P, D], f32, tag="da")
            b = dpool.tile([P, D], f32, tag="db")
            nc.sync.dma_start(a[:], x_slab[si * P:(si + 1) * P, :])
            nc.sync.dma_start(b[:], w_tgt[si * P:(si + 1) * P, :])
            prod = dpool.tile([P, D], f32, tag="dp")
            # NOTE: tensor_tensor_reduce (fused) crashes the device on this
            # runtime path — use separate mult + reduce instead.
            nc.vector.tensor_tensor(out=prod[:], in0=a[:], in1=b[:],
                                    op=Alu.mult)
            nc.vector.reduce_sum(out=D_sb[:, si:si + 1], in_=prod[:],
                                 axis=AX, op=Alu.add)
        nc.sync.dma_start(d_in[:].rearrange("f p -> p f"), D_sb[:])
        import os as _os
        if _os.environ.get("K_NO_COLL"):
            for rep in range(N_CORES):
                nc.gpsimd.dma_start(
                    d_out[rep * SLAB_TI:(rep + 1) * SLAB_TI, :], d_in[:])
        else:
            nc.gpsimd.collective_compute(
                "AllGather", Alu.bypass, replica_groups=rg,
                ins=[d_in[:]], outs=[d_out[:]])

        # ---- main vocab-parallel logit pass ----
        import os as _os2
        NTI_RUN = int(_os2.environ.get("K_NTI", N_TI))
        if NTI_RUN < N_TI:
            nc.vector.memset(S_sb[:], 1.0)
        KP = KT // 2  # DoubleRow processes two 128-deep k-tiles per matmul
        for ti in range(NTI_RUN):
            x = xpool.tile([P, KT, P], f8, tag="x")
            nc.sync.dma_start(x[:], xT[ti])
            acc = apool.tile([P, N_VC], f32, tag="acc")
            for vc in range(N_VC):
                pt = psum.tile([P, VC], f32, tag="pt")
                for j in range(KP):
                    nc.tensor.matmul(
                        pt[:], x[:, 2 * j:2 * j + 2, :],
                        w_sb[:, 2 * j:2 * j + 2, vc * VC:(vc + 1) * VC],
                        start=(j == 0), stop=(j == KP - 1), perf_mode=DR)
                esc = epool.tile([P, VC], f32, tag="esc")
                nc.scalar.activation(
                    esc[:], pt[:], Act.Exp, bias=zero_b[:],
                    scale=1.0 / W_SCALE, accum_out=acc[:, vc:vc + 1])
            nc.vector.reduce_sum(
                out=S_sb[:, ti:ti + 1], in_=acc[:], axis=AX, op=Alu.add)
        nc.sync.dma_start(se_in[:].rearrange("f p -> p f"), S_sb[:])
        if _os.environ.get("K_NO_COLL"):
            nc.gpsimd.dma_start(se_out[:], se_in[:])
        else:
            nc.gpsimd.collective_compute(
                "AllReduce", Alu.add, replica_groups=rg,
                ins=[se_in[:]], outs=[se_out[:]])

        # ---- final reduction (identical on every core) ----
        sv = fin.tile([P, N_TI], f32)
        dv = fin.tile([P, N_TI], f32)
        tv = fin.tile([P, N_TI], i32)
        nc.sync.dma_start(sv[:], se_out[:].rearrange("f p -> p f"))
        nc.sync.dma_start(dv[:], d_out[:].rearrange("f p -> p f"))
        nc.sync.dma_start(tv[:], tgts[:].rearrange("(f p) -> p f", p=P))

        lse = fin.tile([P, N_TI], f32)
        nc.scalar.activation(lse[:], sv[:], Act.Ln, bias=zero_b[:])
        validf = fin.tile([P, N_TI], f32)
        nc.vector.tensor_scalar(
            validf[:], tv[:], IGNORE, None, op0=Alu.not_equal)
        tl = fin.tile([P, N_TI], f32)
        nc.vector.tensor_tensor(
            out=tl[:], in0=lse[:], in1=dv[:], op=Alu.subtract)
        nc.vector.tensor_tensor(
            out=tl[:], in0=tl[:], in1=validf[:], op=Alu.mult)

        HB = N_TI // 2  # batch boundary in free dim
        stats = fin.tile([P, 4], f32)
        nc.vector.reduce_sum(out=stats[:, 0:1], in_=tl[:, :HB], axis=AX,
                             op=Alu.add)
        nc.vector.reduce_sum(out=stats[:, 1:2], in_=tl[:, HB:], axis=AX,
                             op=Alu.add)
        nc.vector.reduce_sum(out=stats[:, 2:3], in_=validf[:, :HB], axis=AX,
                             op=Alu.add)
        nc.vector.reduce_sum(out=stats[:, 3:4], in_=validf[:, HB:], axis=AX,
                             op=Alu.add)

        pfin = psfin.tile([1, 4], f32)
        nc.tensor.matmul(pfin[:], ones[:], stats[:], start=True, stop=True)
        tot = fin.tile([1, 4], f32)
        nc.vector.tensor_copy(out=tot[:], in_=pfin[:])

        sq = fin.tile([1, 2], f32)
        nc.scalar.activation(sq[:], tot[:, 2:4], Act.Sqrt, bias=eps_b[:1, :])
        r = fin.tile([1, 2], f32)
        nc.vector.reciprocal(r[:], sq[:])
        la = fin.tile([1, 2], f32)
        nc.vector.tensor_tensor(out=la[:], in0=tot[:, 0:2], in1=r[:],
                                op=Alu.mult)
        nb = fin.tile([1, 2], f32)
        nc.vector.tensor_tensor(out=nb[:], in0=tot[:, 2:4], in1=r[:],
                                op=Alu.mult)
        las = fin.tile([1, 1], f32)
        nc.vector.reduce_sum(out=las[:], in_=la[:], axis=AX, op=Alu.add)
        nbs = fin.tile([1, 1], f32)
        nc.vector.reduce_sum(out=nbs[:], in_=nb[:], axis=AX, op=Alu.add)
        inv = fin.tile([1, 1], f32)
        nc.vector.reciprocal(inv[:], nbs[:])
        res = fin.tile([1, 1], f32)
        nc.vector.tensor_tensor(out=res[:], in0=las[:], in1=inv[:],
                                op=Alu.mult)
        nc.sync.dma_start(loss_out[:], res[:])

    nc.compile()
    _CACHE["nc"] = nc
    return nc


def _prep_inputs(outputs, proj_weight, targets):
    import concourse.mybir as mybir
    f8np = mybir.dt.np(mybir.dt.float8e4)
    xf = np.ascontiguousarray(np.asarray(outputs, dtype=np.float32)
                              .reshape(T, D))
    w = np.asarray(proj_weight, dtype=np.float32)
    tgt = np.asarray(targets).astype(np.int32).reshape(T)

    # [N_TI, P(tok), KT, P(d)] -> [N_TI, P(d), KT, P(tok)]
    xT = np.ascontiguousarray(
        xf.reshape(N_TI, P, KT, P).transpose(0, 3, 2, 1)).astype(f8np)

    safe = np.where(tgt == IGNORE, 0, tgt)

    in_maps = []
    for c in range(N_CORES):
        ws = w[c * V_SHARD:(c + 1) * V_SHARD]            # [4000, 1024]
        wTc = np.ascontiguousarray(
            (ws.T * W_SCALE).reshape(KT, P, V_SHARD)
            .transpose(1, 0, 2)).astype(f8np)
        sl = slice(c * T_SLAB, (c + 1) * T_SLAB)
        in_maps.append({
            "xT": xT,
            "wT": wTc,
            "w_tgt": np.ascontiguousarray(w[safe[sl]]),
            "x_slab": np.ascontiguousarray(xf[sl]),
            "tgts": tgt,
        })
    return in_maps


def kernel(outputs, proj_weight, targets):
    from concourse.bass_utils import run_bass_kernel_spmd

    nc = _build()
    in_maps = _prep_inputs(outputs, proj_weight, targets)
    res = run_bass_kernel_spmd(nc, in_maps, core_ids=list(range(N_CORES)))
    loss = np.asarray(res.results[0]["loss"], dtype=np.float32).reshape(())
    return loss



# revision 2
# speedup vs baseline: 1.1232x; 1.1232x over previous
"""Vocab-parallel sqrt-length-weighted cross-entropy loss on 8 NeuronCores.

Strategy (8-way vocab parallel):
  - proj_weight's vocab dim is sharded 8 ways (4000 rows/core). Each core
    computes partial logits for all 8192 tokens against its shard with a
    fp8 DoubleRow PE matmul (stationary = token tile, moving = weight
    columns), accumulating 4 k-pairs into 4-bank-wide PSUM groups.
    exp + row-sum happen at PSUM eviction as ONE wide Scalar-engine
    activation per 4-bank group (2048 / 1952 columns) with accum_out,
    so the Scalar engine runs ~2x fewer, ~4x wider instructions than a
    per-500-col eviction would.
  - Each core also computes the exact target logit for its 1024-token
    slab as bf16 DVE dot products (host gathers the target weight rows).
    The dot results are placed into this core's 8 columns of a [128, 64]
    token-layout tile (zero elsewhere via a host-provided mask).
  - ONE AllReduce over a single [128, 128] f32 buffer (sum-exp in cols
    0:64, masked target dots in cols 64:128) replaces the baseline's
    AllGather + AllReduce; all layouts are partition-major so no
    transposing (4-byte-packet) DMAs exist anywhere.
  - Every core then computes the final weighted reduction identically.
"""

import numpy as np
import ml_dtypes

B, S, D, V = 2, 4096, 1024, 32000
N_CORES = 8
P = 128
T = B * S                # 8192 tokens
KT = D // P              # 8 contraction tiles
KP = KT // 2             # 4 DoubleRow k-pairs
N_TI = T // P            # 64 token tiles
V_SHARD = V // N_CORES   # 4000
T_SLAB = T // N_CORES    # 1024
SLAB_TI = T_SLAB // P    # 8
IGNORE = -100
EPS = 1e-8
W_SCALE = 32.0           # pre-scale for fp8 weights (w ~ N(0, 1/1024))

# PSUM groups: 2 groups of 4 banks. Blocks are (offset-in-group, width,
# vocab-offset); every block is <=512 wide and bank-aligned so each
# matmul output stays inside one PSUM bank. Group widths 2048 / 1952
# leave garbage only past the activation's read range.
GROUPS = [
    [(0, 512, 0), (512, 512, 512), (1024, 512, 1024), (1536, 512, 1536)],
    [(0, 512, 2048), (512, 512, 2560), (1024, 512, 3072), (1536, 416, 3584)],
]
G_WIDTH = [2048, 1952]

_CACHE = {}


def _build():
    if "nc" in _CACHE:
        return _CACHE["nc"]

    from contextlib import ExitStack

    import concourse.bacc as bacc
    import concourse.mybir as mybir
    import concourse.tile as tile

    f32 = mybir.dt.float32
    bf16 = mybir.dt.bfloat16
    f8 = mybir.dt.float8e4
    i32 = mybir.dt.int32
    Alu = mybir.AluOpType
    Act = mybir.ActivationFunctionType
    AX = mybir.AxisListType.X
    DR = mybir.MatmulPerfMode.DoubleRow

    nc = bacc.Bacc("TRN2", target_bir_lowering=False, debug=False,
                   num_devices=N_CORES)

    # Inputs (per core): pre-tiled on host.
    # xT[ti, p, k, t] = outputs_flat[ti*128 + t, k*128 + p]  (fp8 e4m3)
    xT = nc.dram_tensor("xT", [N_TI, P, KT, P], f8, kind="ExternalInput")
    # wT[p, k, v] = W_SCALE * w_shard[v, k*128 + p]  (fp8 e4m3)
    wT = nc.dram_tensor("wT", [P, KT, V_SHARD], f8, kind="ExternalInput")
    # gathered target weight rows + activations for this core's 1024-token
    # slab (bf16; the dot is accumulated in f32 on the DVE)
    w_tgt = nc.dram_tensor("w_tgt", [T_SLAB, D], bf16, kind="ExternalInput")
    x_slab = nc.dram_tensor("x_slab", [T_SLAB, D], bf16, kind="ExternalInput")
    # full targets pre-tiled: tgtT[p, ti] = targets[ti*128 + p]
    tgtT = nc.dram_tensor("tgtT", [P, N_TI], i32, kind="ExternalInput")
    # mask[p, ti] = 1.0 iff token ti*128+p belongs to this core's slab
    mask = nc.dram_tensor("mask", [P, N_TI], f32, kind="ExternalInput")
    loss_out = nc.dram_tensor("loss", [1, 1], f32, kind="ExternalOutput")

    # Single fused collective bounce buffer: cols 0:64 local sum-exp,
    # cols 64:128 masked target dots. Same [p, ti] layout on every core,
    # so AllReduce-add yields global sum-exp AND the full dot vector.
    ar_in = nc.dram_tensor("ar_in", [P, 2 * N_TI], f32)
    ar_out = nc.dram_tensor("ar_out", [P, 2 * N_TI], f32, addr_space="Shared")

    rg = [list(range(N_CORES))]

    with tile.TileContext(nc) as tc, ExitStack() as ctx:
        const = ctx.enter_context(tc.tile_pool(name="const", bufs=1))
        wpool = ctx.enter_context(tc.tile_pool(name="wpool", bufs=1))
        xpool = ctx.enter_context(tc.tile_pool(name="xpool", bufs=3))
        psum = ctx.enter_context(
            tc.tile_pool(name="psum", bufs=2, space="PSUM"))
        epool = ctx.enter_context(tc.tile_pool(name="epool", bufs=2))
        apool = ctx.enter_context(tc.tile_pool(name="apool", bufs=3))
        dpool = ctx.enter_context(tc.tile_pool(name="dpool", bufs=2))
        spool = ctx.enter_context(tc.tile_pool(name="spool", bufs=1))
        fin = ctx.enter_context(tc.tile_pool(name="fin", bufs=1))

        zero_b = const.tile([P, 1], f32)
        nc.vector.memset(zero_b[:], 0.0)
        eps_b = const.tile([P, 1], f32)
        nc.vector.memset(eps_b[:], EPS)
        ones = const.tile([P, 1], f32)
        nc.vector.memset(ones[:], 1.0)

        # Resident weight shard, loaded in 4 k-pair chunks (1 MB each) so
        # the first matmuls can start after ~3 us instead of ~12.
        w_sb = wpool.tile([P, KT, V_SHARD], f8)
        for j in range(KP):
            nc.sync.dma_start(w_sb[:, 2 * j:2 * j + 2, :],
                              wT[:, 2 * j:2 * j + 2, :])

        mask_sb = const.tile([P, N_TI], f32)
        nc.gpsimd.dma_start(mask_sb[:], mask[:])
        tv = fin.tile([P, N_TI], i32)
        nc.gpsimd.dma_start(tv[:], tgtT[:])

        # sf: cols 0:64 = per-token local sum-exp, cols 64:128 = masked dots
        sf = spool.tile([P, 2 * N_TI], f32)
        D_sb = spool.tile([P, SLAB_TI], f32)  # per-token target dot (slab)

        # ---- main vocab-parallel logit pass, dot-product work interleaved ----
        for ti in range(N_TI):
            x = xpool.tile([P, KT, P], f8, tag="x")
            nc.sync.dma_start(x[:], xT[ti])
            acc = apool.tile([P, 2], f32, tag="acc")
            for g, blocks in enumerate(GROUPS):
                pt = psum.tile([P, 2048], f32, tag="pt")
                for off, wd, voff in blocks:
                    for j in range(KP):
                        nc.tensor.matmul(
                            pt[:, off:off + wd], x[:, 2 * j:2 * j + 2, :],
                            w_sb[:, 2 * j:2 * j + 2, voff:voff + wd],
                            start=(j == 0), stop=(j == KP - 1), perf_mode=DR)
                esc = epool.tile([P, 2048], bf16, tag="esc")
                gw = G_WIDTH[g]
                nc.scalar.activation(
                    esc[:, :gw], pt[:, :gw], Act.Exp, bias=zero_b[:],
                    scale=1.0 / W_SCALE, accum_out=acc[:, g:g + 1])
            nc.vector.tensor_tensor(out=sf[:, ti:ti + 1], in0=acc[:, 0:1],
                                    in1=acc[:, 1:2], op=Alu.add)

            # target-logit dots (DVE, bf16 in / f32 accum), spread across
            # the loop so DMA + DVE work overlaps the matmul stream
            if ti >= 4 and ti % 6 == 4 and (ti - 4) // 6 < SLAB_TI:
                si = (ti - 4) // 6
                a = dpool.tile([P, D], bf16, tag="da")
                b = dpool.tile([P, D], bf16, tag="db")
                nc.gpsimd.dma_start(a[:], x_slab[si * P:(si + 1) * P, :])
                nc.gpsimd.dma_start(b[:], w_tgt[si * P:(si + 1) * P, :])
                prod = dpool.tile([P, D], f32, tag="dp")
                # NOTE: fused tensor_tensor_reduce crashes the device on
                # this runtime path — separate mult + reduce instead.
                nc.vector.tensor_tensor(out=prod[:], in0=a[:], in1=b[:],
                                        op=Alu.mult)
                nc.vector.reduce_sum(out=D_sb[:, si:si + 1], in_=prod[:],
                                     axis=AX, op=Alu.add)
            if ti == 58:
                # place my slab's dots into my 8 columns, zero elsewhere
                for k in range(N_CORES):
                    nc.vector.tensor_tensor(
                        out=sf[:, N_TI + k * SLAB_TI:N_TI + (k + 1) * SLAB_TI],
                        in0=D_sb[:], in1=mask_sb[:, k * SLAB_TI:(k + 1) * SLAB_TI],
                        op=Alu.mult)

        nc.gpsimd.dma_start(ar_in[:], sf[:])
        nc.gpsimd.collective_compute(
            "AllReduce", Alu.add, replica_groups=rg,
            ins=[ar_in[:]], outs=[ar_out[:]])

        # ---- final reduction (identical on every core) ----
        gl = fin.tile([P, 2 * N_TI], f32)
        nc.gpsimd.dma_start(gl[:], ar_out[:])
        sv = gl[:, 0:N_TI]
        dv = gl[:, N_TI:2 * N_TI]

        lse = fin.tile([P, N_TI], f32)
        nc.scalar.activation(lse[:], sv, Act.Ln, bias=zero_b[:])
        validf = fin.tile([P, N_TI], f32)
        nc.vector.tensor_scalar(
            validf[:], tv[:], IGNORE, None, op0=Alu.not_equal)
        tl = fin.tile([P, N_TI], f32)
        nc.vector.tensor_tensor(
            out=tl[:], in0=lse[:], in1=dv, op=Alu.subtract)
        nc.vector.tensor_tensor(
            out=tl[:], in0=tl[:], in1=validf[:], op=Alu.mult)

        HB = N_TI // 2  # batch boundary in free dim
        stats = fin.tile([P, 4], f32)
        nc.vector.reduce_sum(out=stats[:, 0:1], in_=tl[:, :HB], axis=AX,
                             op=Alu.add)
        nc.vector.reduce_sum(out=stats[:, 1:2], in_=tl[:, HB:], axis=AX,
                             op=Alu.add)
        nc.vector.reduce_sum(out=stats[:, 2:3], in_=validf[:, :HB], axis=AX,
                             op=Alu.add)
        nc.vector.reduce_sum(out=stats[:, 3:4], in_=validf[:, HB:], axis=AX,
                             op=Alu.add)

        pfin = psum.tile([1, 4], f32, tag="pt")
        nc.tensor.matmul(pfin[:], ones[:], stats[:], start=True, stop=True)
        tot = fin.tile([1, 4], f32)
        nc.vector.tensor_copy(out=tot[:], in_=pfin[:])

        sq = fin.tile([1, 2], f32)
        nc.scalar.activation(sq[:], tot[:, 2:4], Act.Sqrt, bias=eps_b[:1, :])
        r = fin.tile([1, 2], f32)
        nc.vector.reciprocal(r[:], sq[:])
        la = fin.tile([1, 2], f32)
        nc.vector.tensor_tensor(out=la[:], in0=tot[:, 0:2], in1=r[:],
                                op=Alu.mult)
        nb = fin.tile([1, 2], f32)
        nc.vector.tensor_tensor(out=nb[:], in0=tot[:, 2:4], in1=r[:],
                                op=Alu.mult)
        las = fin.tile([1, 1], f32)
        nc.vector.reduce_sum(out=las[:], in_=la[:], axis=AX, op=Alu.add)
        nbs = fin.tile([1, 1], f32)
        nc.vector.reduce_sum(out=nbs[:], in_=nb[:], axis=AX, op=Alu.add)
        inv = fin.tile([1, 1], f32)
        nc.vector.reciprocal(inv[:], nbs[:])
        res = fin.tile([1, 1], f32)
        nc.vector.tensor_tensor(out=res[:], in0=las[:], in1=inv[:],
                                op=Alu.mult)
        nc.sync.dma_start(loss_out[:], res[:])

    nc.compile()
    _CACHE["nc"] = nc
    return nc


def _prep_inputs(outputs, proj_weight, targets):
    import concourse.mybir as mybir
    f8np = mybir.dt.np(mybir.dt.float8e4)
    bf16np = ml_dtypes.bfloat16
    xf = np.ascontiguousarray(np.asarray(outputs, dtype=np.float32)
                              .reshape(T, D))
    w = np.asarray(proj_weight, dtype=np.float32)
    tgt = np.asarray(targets).astype(np.int32).reshape(T)

    # [N_TI, P(tok), KT, P(d)] -> [N_TI, P(d), KT, P(tok)]
    xT = np.ascontiguousarray(
        xf.reshape(N_TI, P, KT, P).transpose(0, 3, 2, 1)).astype(f8np)

    safe = np.where(tgt == IGNORE, 0, tgt)
    tgtT = np.ascontiguousarray(tgt.reshape(N_TI, P).T)

    in_maps = []
    for c in range(N_CORES):
        ws = w[c * V_SHARD:(c + 1) * V_SHARD]            # [4000, 1024]
        wTc = np.ascontiguousarray(
            (ws.T * W_SCALE).reshape(KT, P, V_SHARD)
            .transpose(1, 0, 2)).astype(f8np)
        sl = slice(c * T_SLAB, (c + 1) * T_SLAB)
        mk = np.zeros((P, N_TI), dtype=np.float32)
        mk[:, c * SLAB_TI:(c + 1) * SLAB_TI] = 1.0
        in_maps.append({
            "xT": xT,
            "wT": wTc,
            "w_tgt": np.ascontiguousarray(w[safe[sl]]).astype(bf16np),
            "x_slab": np.ascontiguousarray(xf[sl]).astype(bf16np),
            "tgtT": tgtT,
            "mask": mk,
        })
    return in_maps


def kernel(outputs, proj_weight, targets):
    from concourse.bass_utils import run_bass_kernel_spmd

    nc = _build()
    in_maps = _prep_inputs(outputs, proj_weight, targets)
    res = run_bass_kernel_spmd(nc, in_maps, core_ids=list(range(N_CORES)))
    loss = np.asarray(res.results[0]["loss"], dtype=np.float32).reshape(())
    return loss


# revision 3
# speedup vs baseline: 1.1759x; 1.0469x over previous
"""Vocab-parallel sqrt-length-weighted cross-entropy loss on 8 NeuronCores.

Strategy (8-way vocab parallel):
  - proj_weight's vocab dim is sharded 8 ways (4000 rows/core). Each core
    computes partial logits for all token tiles against its shard with a
    fp8 DoubleRow PE matmul (stationary = token tile, moving = weight
    columns), accumulating 4 k-pairs into 4-bank-wide PSUM groups.
    exp + row-sum happen at PSUM eviction as ONE wide Scalar-engine
    activation per 4-bank group (2048 / 1952 columns) with accum_out.
  - Token tiles that are fully ignore_index (the prompt prefix) are
    skipped entirely; the skip set is derived from targets at prep time.
  - Each core computes the exact target logit for its 1024-token slab as
    bf16 DVE dot products (host gathers the target weight rows), placed
    into this core's 8 columns of a [128, 64] token-layout tile (zero
    elsewhere via a host-provided mask).
  - Collectives: a zero-byte warm-up AllReduce absorbs the cold-start
    cost early; the real reduction is split so the bulk (dots + 48 of 64
    sum-exp columns) all-reduces while the last token tiles still
    compute, leaving only a tiny 8KB AllReduce on the critical tail.
  - Every core then computes the final weighted reduction identically.
    All layouts are partition-major: no transposing DMAs anywhere.
"""

import numpy as np
import ml_dtypes

B, S, D, V = 2, 4096, 1024, 32000
N_CORES = 8
P = 128
T = B * S                # 8192 tokens
KT = D // P              # 8 contraction tiles
KP = KT // 2             # 4 DoubleRow k-pairs
N_TI = T // P            # 64 token tiles
V_SHARD = V // N_CORES   # 4000
VH = 2048                # vocab half (g0 columns; g1 = V_SHARD - VH)
T_SLAB = T // N_CORES    # 1024
SLAB_TI = T_SLAB // P    # 8
IGNORE = -100
EPS = 1e-8
W_SCALE = 32.0           # pre-scale for fp8 weights (w ~ N(0, 1/1024))

# sf/collective layout: cols 0:64 = masked target dots, 64:128 = sum-exp
DCOL = 0
SCOL = N_TI
AR1_W = 112              # dots + sum-exp of token tiles 0..47
AR1_TI = 47              # last token tile covered by AR#1

# PSUM groups: 2 groups of 4 banks. Blocks are (offset-in-group, width,
# vocab-offset); every block is <=512 wide and bank-aligned so each
# matmul output stays inside one PSUM bank. Group widths 2048 / 1952
# leave garbage only past the activation's read range.
GROUPS = [
    [(0, 512, 0), (512, 512, 512), (1024, 512, 1024), (1536, 512, 1536)],
    [(0, 512, 2048), (512, 512, 2560), (1024, 512, 3072), (1536, 416, 3584)],
]
G_WIDTH = [2048, 1952]

_CACHE = {}


def _build(skip_tiles=frozenset()):
    key = ("nc", skip_tiles)
    if key in _CACHE:
        return _CACHE[key]

    from contextlib import ExitStack

    import concourse.bacc as bacc
    import concourse.mybir as mybir
    import concourse.tile as tile

    f32 = mybir.dt.float32
    bf16 = mybir.dt.bfloat16
    f8 = mybir.dt.float8e4
    i32 = mybir.dt.int32
    Alu = mybir.AluOpType
    Act = mybir.ActivationFunctionType
    AX = mybir.AxisListType.X
    DR = mybir.MatmulPerfMode.DoubleRow

    nc = bacc.Bacc("TRN2", target_bir_lowering=False, debug=False,
                   num_devices=N_CORES)

    # Inputs (per core): pre-tiled on host.
    # xT[ti, p, k, t] = outputs_flat[ti*128 + t, k*128 + p]  (fp8 e4m3)
    xT = nc.dram_tensor("xT", [N_TI, P, KT, P], f8, kind="ExternalInput")
    # wT[p, k, v] = W_SCALE * w_shard[v, k*128 + p]  (fp8 e4m3)
    wT = nc.dram_tensor("wT", [P, KT, V_SHARD], f8, kind="ExternalInput")
    # gathered target weight rows + activations for this core's 1024-token
    # slab (bf16; the dot is accumulated in f32 on the DVE)
    w_tgt = nc.dram_tensor("w_tgt", [T_SLAB, D], bf16, kind="ExternalInput")
    x_slab = nc.dram_tensor("x_slab", [T_SLAB, D], bf16, kind="ExternalInput")
    # full targets pre-tiled: tgtT[p, ti] = targets[ti*128 + p]
    tgtT = nc.dram_tensor("tgtT", [P, N_TI], i32, kind="ExternalInput")
    # mask[p, ti] = 1.0 iff token ti*128+p belongs to this core's slab
    mask = nc.dram_tensor("mask", [P, N_TI], f32, kind="ExternalInput")
    loss_out = nc.dram_tensor("loss", [1, 1], f32, kind="ExternalOutput")

    # Collective bounce buffers. ar1 carries the dots plus sum-exp for
    # token tiles 0..47 (all-reduced while the tail tiles still compute);
    # ar2 carries sum-exp for tiles 48..63. wu warms up the CC path.
    ar1_in = nc.dram_tensor("ar1_in", [P, AR1_W], f32)
    ar1_out = nc.dram_tensor("ar1_out", [P, AR1_W], f32, addr_space="Shared")
    ar2_in = nc.dram_tensor("ar2_in", [P, 2 * N_TI - AR1_W], f32)
    ar2_out = nc.dram_tensor("ar2_out", [P, 2 * N_TI - AR1_W], f32,
                             addr_space="Shared")
    wu_in = nc.dram_tensor("wu_in", [1, 16], f32)
    wu_out = nc.dram_tensor("wu_out", [1, 16], f32, addr_space="Shared")

    rg = [list(range(N_CORES))]
    active = [ti for ti in range(N_TI) if ti not in skip_tiles]

    with tile.TileContext(nc) as tc, ExitStack() as ctx:
        const = ctx.enter_context(tc.tile_pool(name="const", bufs=1))
        wpool = ctx.enter_context(tc.tile_pool(name="wpool", bufs=1))
        xpool = ctx.enter_context(tc.tile_pool(name="xpool", bufs=3))
        psum = ctx.enter_context(
            tc.tile_pool(name="psum", bufs=2, space="PSUM"))
        epool = ctx.enter_context(tc.tile_pool(name="epool", bufs=2))
        apool = ctx.enter_context(tc.tile_pool(name="apool", bufs=3))
        dpool = ctx.enter_context(tc.tile_pool(name="dpool", bufs=2))
        spool = ctx.enter_context(tc.tile_pool(name="spool", bufs=1))
        fin = ctx.enter_context(tc.tile_pool(name="fin", bufs=1))

        # Warm up the collective path immediately (contents irrelevant).
        nc.gpsimd.collective_compute(
            "AllReduce", Alu.add, replica_groups=rg,
            ins=[wu_in[:]], outs=[wu_out[:]])

        zero_b = const.tile([P, 1], f32)
        nc.vector.memset(zero_b[:], 0.0)
        eps_b = const.tile([P, 1], f32)
        nc.vector.memset(eps_b[:], EPS)
        ones = const.tile([P, 1], f32)
        nc.vector.memset(ones[:], 1.0)

        # Resident weight shard. Load order: g0 k-pair chunks first with
        # the first token tile right behind them, so the PE starts after
        # ~1.5 MB instead of the full 4.1 MB.
        w_sb = wpool.tile([P, KT, V_SHARD], f8)
        nc.sync.dma_start(w_sb[:, 0:2, :VH], wT[:, 0:2, :VH])
        xtile0 = xpool.tile([P, KT, P], f8, tag="x", name="x")
        nc.sync.dma_start(xtile0[:], xT[active[0]])
        for j in range(1, KP):
            nc.sync.dma_start(w_sb[:, 2 * j:2 * j + 2, :VH],
                              wT[:, 2 * j:2 * j + 2, :VH])
        for j in range(KP):
            nc.sync.dma_start(w_sb[:, 2 * j:2 * j + 2, VH:],
                              wT[:, 2 * j:2 * j + 2, VH:])

        mask_sb = const.tile([P, N_TI], f32)
        nc.gpsimd.dma_start(mask_sb[:], mask[:])
        tv = fin.tile([P, N_TI], i32)
        nc.gpsimd.dma_start(tv[:], tgtT[:])

        # sf: cols 0:64 = masked dots, cols 64:128 = per-token local sum-exp
        sf = spool.tile([P, 2 * N_TI], f32)
        D_sb = spool.tile([P, SLAB_TI], f32)  # per-token target dot (slab)
        for ti in skip_tiles:
            nc.vector.memset(sf[:, SCOL + ti:SCOL + ti + 1], 1.0)

        # ---- main vocab-parallel logit pass, dot work interleaved ----
        for idx, ti in enumerate(active):
            if idx == 0:
                x = xtile0
            else:
                x = xpool.tile([P, KT, P], f8, tag="x")
                nc.sync.dma_start(x[:], xT[ti])
            acc = apool.tile([P, 2], f32, tag="acc")
            for g, blocks in enumerate(GROUPS):
                pt = psum.tile([P, 2048], f32, tag="pt")
                for off, wd, voff in blocks:
                    for j in range(KP):
                        nc.tensor.matmul(
                            pt[:, off:off + wd], x[:, 2 * j:2 * j + 2, :],
                            w_sb[:, 2 * j:2 * j + 2, voff:voff + wd],
                            start=(j == 0), stop=(j == KP - 1), perf_mode=DR)
                esc = epool.tile([P, 2048], bf16, tag="esc")
                gw = G_WIDTH[g]
                nc.scalar.activation(
                    esc[:, :gw], pt[:, :gw], Act.Exp, bias=zero_b[:],
                    scale=1.0 / W_SCALE, accum_out=acc[:, g:g + 1])
            nc.vector.tensor_tensor(out=sf[:, SCOL + ti:SCOL + ti + 1],
                                    in0=acc[:, 0:1], in1=acc[:, 1:2],
                                    op=Alu.add)

            # target-logit dots (DVE, bf16 in / f32 accum). Their DMAs are
            # issued from the Scalar queue so they trail the weight/x head
            # transfers instead of racing them (GpSimd runs ahead).
            if idx >= 4 and idx % 2 == 0 and (idx - 4) // 2 < SLAB_TI:
                si = (idx - 4) // 2
                a = dpool.tile([P, D], bf16, tag="da")
                b = dpool.tile([P, D], bf16, tag="db")
                nc.scalar.dma_start(a[:], x_slab[si * P:(si + 1) * P, :])
                nc.scalar.dma_start(b[:], w_tgt[si * P:(si + 1) * P, :])
                prod = dpool.tile([P, D], f32, tag="dp")
                # NOTE: fused tensor_tensor_reduce crashes the device on
                # this runtime path — separate mult + reduce instead.
                nc.vector.tensor_tensor(out=prod[:], in0=a[:], in1=b[:],
                                        op=Alu.mult)
                nc.vector.reduce_sum(out=D_sb[:, si:si + 1], in_=prod[:],
                                     axis=AX, op=Alu.add)
            if idx == 22:
                # place my slab's dots into my 8 columns, zero elsewhere
                for k in range(N_CORES):
                    nc.vector.tensor_tensor(
                        out=sf[:, DCOL + k * SLAB_TI:DCOL + (k + 1) * SLAB_TI],
                        in0=D_sb[:],
                        in1=mask_sb[:, k * SLAB_TI:(k + 1) * SLAB_TI],
                        op=Alu.mult)
            if ti == AR1_TI:
                # bulk reduction: dots + sum-exp of tiles 0..47, hidden
                # behind the remaining token tiles' compute
                nc.gpsimd.dma_start(ar1_in[:], sf[:, :AR1_W])
                nc.gpsimd.collective_compute(
                    "AllReduce", Alu.add, replica_groups=rg,
                    ins=[ar1_in[:]], outs=[ar1_out[:]])

        gl = fin.tile([P, 2 * N_TI], f32)
        nc.gpsimd.dma_start(gl[:, :AR1_W], ar1_out[:])

        nc.gpsimd.dma_start(ar2_in[:], sf[:, AR1_W:])
        nc.gpsimd.collective_compute(
            "AllReduce", Alu.add, replica_groups=rg,
            ins=[ar2_in[:]], outs=[ar2_out[:]])
        nc.gpsimd.dma_start(gl[:, AR1_W:], ar2_out[:])

        # ---- final reduction (identical on every core) ----
        dv = gl[:, DCOL:DCOL + N_TI]
        sv = gl[:, SCOL:SCOL + N_TI]

        lse = fin.tile([P, N_TI], f32)
        nc.scalar.activation(lse[:], sv, Act.Ln, bias=zero_b[:])
        validf = fin.tile([P, N_TI], f32)
        nc.vector.tensor_scalar(
            validf[:], tv[:], IGNORE, None, op0=Alu.not_equal)
        tl = fin.tile([P, N_TI], f32)
        nc.vector.tensor_tensor(
            out=tl[:], in0=lse[:], in1=dv, op=Alu.subtract)
        nc.vector.tensor_tensor(
            out=tl[:], in0=tl[:], in1=validf[:], op=Alu.mult)

        HB = N_TI // 2  # batch boundary in free dim
        stats = fin.tile([P, 4], f32)
        nc.vector.reduce_sum(out=stats[:, 0:1], in_=tl[:, :HB], axis=AX,
                             op=Alu.add)
        nc.vector.reduce_sum(out=stats[:, 1:2], in_=tl[:, HB:], axis=AX,
                             op=Alu.add)
        nc.vector.reduce_sum(out=stats[:, 2:3], in_=validf[:, :HB], axis=AX,
                             op=Alu.add)
        nc.vector.reduce_sum(out=stats[:, 3:4], in_=validf[:, HB:], axis=AX,
                             op=Alu.add)

        pfin = psum.tile([1, 4], f32, tag="pt")
        nc.tensor.matmul(pfin[:], ones[:], stats[:], start=True, stop=True)
        tot = fin.tile([1, 4], f32)
        nc.vector.tensor_copy(out=tot[:], in_=pfin[:])

        # sqrt(x) = exp(0.5*ln(x)) — stays on the ln/exp activation table,
        # avoiding a 1.3us table swap on the critical tail
        lnv = fin.tile([1, 2], f32)
        nc.scalar.activation(lnv[:], tot[:, 2:4], Act.Ln, bias=eps_b[:1, :])
        sq = fin.tile([1, 2], f32)
        nc.scalar.activation(sq[:], lnv[:], Act.Exp, bias=zero_b[:1, :],
                             scale=0.5)
        r = fin.tile([1, 2], f32)
        nc.vector.reciprocal(r[:], sq[:])
        la = fin.tile([1, 2], f32)
        nc.vector.tensor_tensor(out=la[:], in0=tot[:, 0:2], in1=r[:],
                                op=Alu.mult)
        nb = fin.tile([1, 2], f32)
        nc.vector.tensor_tensor(out=nb[:], in0=tot[:, 2:4], in1=r[:],
                                op=Alu.mult)
        las = fin.tile([1, 1], f32)
        nc.vector.reduce_sum(out=las[:], in_=la[:], axis=AX, op=Alu.add)
        nbs = fin.tile([1, 1], f32)
        nc.vector.reduce_sum(out=nbs[:], in_=nb[:], axis=AX, op=Alu.add)
        inv = fin.tile([1, 1], f32)
        nc.vector.reciprocal(inv[:], nbs[:])
        res = fin.tile([1, 1], f32)
        nc.vector.tensor_tensor(out=res[:], in0=las[:], in1=inv[:],
                                op=Alu.mult)
        nc.sync.dma_start(loss_out[:], res[:])

    nc.compile()
    _CACHE[key] = nc
    return nc


def _skip_tiles(tgt):
    return frozenset(
        ti for ti in range(N_TI)
        if np.all(tgt[ti * P:(ti + 1) * P] == IGNORE))


def _prep_inputs(outputs, proj_weight, targets):
    import concourse.mybir as mybir
    f8np = mybir.dt.np(mybir.dt.float8e4)
    bf16np = ml_dtypes.bfloat16
    xf = np.ascontiguousarray(np.asarray(outputs, dtype=np.float32)
                              .reshape(T, D))
    w = np.asarray(proj_weight, dtype=np.float32)
    tgt = np.asarray(targets).astype(np.int32).reshape(T)

    # [N_TI, P(tok), KT, P(d)] -> [N_TI, P(d), KT, P(tok)]
    xT = np.ascontiguousarray(
        xf.reshape(N_TI, P, KT, P).transpose(0, 3, 2, 1)).astype(f8np)

    safe = np.where(tgt == IGNORE, 0, tgt)
    tgtT = np.ascontiguousarray(tgt.reshape(N_TI, P).T)

    in_maps = []
    for c in range(N_CORES):
        ws = w[c * V_SHARD:(c + 1) * V_SHARD]            # [4000, 1024]
        wTc = np.ascontiguousarray(
            (ws.T * W_SCALE).reshape(KT, P, V_SHARD)
            .transpose(1, 0, 2)).astype(f8np)
        sl = slice(c * T_SLAB, (c + 1) * T_SLAB)
        mk = np.zeros((P, N_TI), dtype=np.float32)
        mk[:, c * SLAB_TI:(c + 1) * SLAB_TI] = 1.0
        in_maps.append({
            "xT": xT,
            "wT": wTc,
            "w_tgt": np.ascontiguousarray(w[safe[sl]]).astype(bf16np),
            "x_slab": np.ascontiguousarray(xf[sl]).astype(bf16np),
            "tgtT": tgtT,
            "mask": mk,
        })
    return in_maps


def kernel(outputs, proj_weight, targets):
    from concourse.bass_utils import run_bass_kernel_spmd

    tgt = np.asarray(targets).astype(np.int32).reshape(T)
    nc = _build(_skip_tiles(tgt))
    in_maps = _prep_inputs(outputs, proj_weight, targets)
    res = run_bass_kernel_spmd(nc, in_maps, core_ids=list(range(N_CORES)))
    loss = np.asarray(res.results[0]["loss"], dtype=np.float32).reshape(())
    return loss


# revision 6
# speedup vs baseline: 1.2136x; 1.0321x over previous
"""Vocab-parallel sqrt-length-weighted cross-entropy loss on 8 NeuronCores.

Strategy (8-way vocab parallel):
  - proj_weight's vocab dim is sharded 8 ways (4000 rows/core). Each core
    computes partial logits for all token tiles against its shard with a
    fp8 DoubleRow PE matmul (stationary = token tile, moving = weight
    columns), accumulating 4 k-pairs into 4-bank-wide PSUM groups.
    exp + row-sum happen at PSUM eviction as ONE wide Scalar-engine
    activation per 4-bank group (2048 / 1952 columns) with accum_out.
  - Token tiles that are fully ignore_index (the prompt prefix) are
    skipped entirely; the skip set is derived from targets at prep time.
  - Each core computes the exact target logit for its 1024-token slab as
    bf16 DVE dot products (host gathers the target weight rows), placed
    into this core's 8 columns of a [128, 64] token-layout tile (zero
    elsewhere via a host-provided mask).
  - Collectives: a warm-up AllReduce absorbs the cold-start cost early;
    the real reduction is split so the bulk (dots + 48 of 64 sum-exp
    columns) all-reduces while the last token tiles still compute. A
    second warm-up right before keeps the CC cores hot for the final
    8KB AllReduce, the only collective on the critical tail.
  - Everything not dependent on the last collective (valid counts, the
    sqrt-weight path, log/sub/reduce for tiles 0..47) is computed hidden
    under the matmul stream; the tail is just 16 columns of math.
"""

import numpy as np
import ml_dtypes

B, S, D, V = 2, 4096, 1024, 32000
N_CORES = 8
P = 128
T = B * S                # 8192 tokens
KT = D // P              # 8 contraction tiles
KP = KT // 2             # 4 DoubleRow k-pairs
N_TI = T // P            # 64 token tiles
V_SHARD = V // N_CORES   # 4000
T_SLAB = T // N_CORES    # 1024
SLAB_TI = T_SLAB // P    # 8
IGNORE = -100
EPS = 1e-8
W_SCALE = 32.0           # pre-scale for fp8 weights (w ~ N(0, 1/1024))

# sf/collective layout: cols 0:64 = masked target dots, 64:128 = sum-exp
DCOL = 0
SCOL = N_TI
AR1_W = 112              # dots + sum-exp of token tiles 0..47
AR1_TI = 47              # last token tile covered by AR#1
HB = N_TI // 2           # batch boundary (token tile 32)

# PSUM groups: 2 groups of 4 banks. Blocks are (offset-in-group, width,
# vocab-offset); every block is <=512 wide and bank-aligned so each
# matmul output stays inside one PSUM bank. Group widths 2048 / 1952
# leave garbage only past the activation's read range.
GROUPS = [
    [(0, 512, 0), (512, 512, 512), (1024, 512, 1024), (1536, 512, 1536)],
    [(0, 512, 2048), (512, 512, 2560), (1024, 512, 3072), (1536, 416, 3584)],
]
G_WIDTH = [2048, 1952]
# weight DMA chunk order: vocab quarters, j-pairs within — the PE's
# consumption order, so the first matmul starts after ~0.5 MB
W_QUARTERS = [(0, 1024), (1024, 2048), (2048, 3072), (3072, 4000)]

_CACHE = {}


def _build(skip_tiles=frozenset()):
    key = ("nc", skip_tiles)
    if key in _CACHE:
        return _CACHE[key]

    from contextlib import ExitStack

    import concourse.bacc as bacc
    import concourse.mybir as mybir
    import concourse.tile as tile

    f32 = mybir.dt.float32
    bf16 = mybir.dt.bfloat16
    f8 = mybir.dt.float8e4
    i32 = mybir.dt.int32
    Alu = mybir.AluOpType
    Act = mybir.ActivationFunctionType
    AX = mybir.AxisListType.X
    DR = mybir.MatmulPerfMode.DoubleRow

    nc = bacc.Bacc("TRN2", target_bir_lowering=False, debug=False,
                   num_devices=N_CORES)

    # Inputs (per core): pre-tiled on host.
    # xT[ti, p, k, t] = outputs_flat[ti*128 + t, k*128 + p]  (fp8 e4m3)
    xT = nc.dram_tensor("xT", [N_TI, P, KT, P], f8, kind="ExternalInput")
    # wT[p, k, v] = W_SCALE * w_shard[v, k*128 + p]  (fp8 e4m3)
    wT = nc.dram_tensor("wT", [P, KT, V_SHARD], f8, kind="ExternalInput")
    # gathered target weight rows + activations for this core's 1024-token
    # slab (bf16; the dot is accumulated in f32 on the DVE)
    w_tgt = nc.dram_tensor("w_tgt", [T_SLAB, D], bf16, kind="ExternalInput")
    x_slab = nc.dram_tensor("x_slab", [T_SLAB, D], bf16, kind="ExternalInput")
    # full targets pre-tiled: tgtT[p, ti] = targets[ti*128 + p]
    tgtT = nc.dram_tensor("tgtT", [P, N_TI], i32, kind="ExternalInput")
    # mask[p, ti] = 1.0 iff token ti*128+p belongs to this core's slab
    mask = nc.dram_tensor("mask", [P, N_TI], f32, kind="ExternalInput")
    loss_out = nc.dram_tensor("loss", [1, 1], f32, kind="ExternalOutput")

    # Collective bounce buffers + warm-ups + tiny transpose scratch.
    ar1_in = nc.dram_tensor("ar1_in", [P, AR1_W], f32)
    ar1_out = nc.dram_tensor("ar1_out", [P, AR1_W], f32, addr_space="Shared")
    ar2_in = nc.dram_tensor("ar2_in", [P, 2 * N_TI - AR1_W], f32)
    ar2_out = nc.dram_tensor("ar2_out", [P, 2 * N_TI - AR1_W], f32,
                             addr_space="Shared")
    wu_in = nc.dram_tensor("wu_in", [1, 16], f32)
    wu_out = nc.dram_tensor("wu_out", [1, 16], f32, addr_space="Shared")
    wu2_in = nc.dram_tensor("wu2_in", [1, 16], f32)
    wu2_out = nc.dram_tensor("wu2_out", [1, 16], f32, addr_space="Shared")
    vs_d = nc.dram_tensor("vs_d", [P, 2], f32)
    r_d = nc.dram_tensor("r_d", [2, 1], f32)
    nb_d = nc.dram_tensor("nb_d", [2, 1], f32)

    rg = [list(range(N_CORES))]
    active = [ti for ti in range(N_TI) if ti not in skip_tiles]
    n_act = len(active)
    ar1_trigger = max(t for t in active if t <= AR1_TI)

    with tile.TileContext(nc) as tc, ExitStack() as ctx:
        const = ctx.enter_context(tc.tile_pool(name="const", bufs=1))
        wpool = ctx.enter_context(tc.tile_pool(name="wpool", bufs=1))
        xpool = ctx.enter_context(tc.tile_pool(name="xpool", bufs=3))
        psum = ctx.enter_context(
            tc.tile_pool(name="psum", bufs=2, space="PSUM"))
        epool = ctx.enter_context(tc.tile_pool(name="epool", bufs=2))
        apool = ctx.enter_context(tc.tile_pool(name="apool", bufs=3))
        dpool = ctx.enter_context(tc.tile_pool(name="dpool", bufs=2))
        spool = ctx.enter_context(tc.tile_pool(name="spool", bufs=1))
        fin = ctx.enter_context(tc.tile_pool(name="fin", bufs=1))

        # Warm up the collective path immediately (contents irrelevant).
        nc.gpsimd.collective_compute(
            "AllReduce", Alu.add, replica_groups=rg,
            ins=[wu_in[:]], outs=[wu_out[:]])

        zero_b = const.tile([P, 1], f32)
        nc.vector.memset(zero_b[:], 0.0)
        eps_b = const.tile([P, 1], f32)
        nc.vector.memset(eps_b[:], EPS)
        ones = const.tile([P, 1], f32)
        nc.vector.memset(ones[:], 1.0)

        # Resident weight shard, streamed in PE consumption order.
        w_sb = wpool.tile([P, KT, V_SHARD], f8)
        lo, hi = W_QUARTERS[0]
        nc.sync.dma_start(w_sb[:, 0:2, lo:hi], wT[:, 0:2, lo:hi])
        xtile0 = xpool.tile([P, KT, P], f8, tag="x", name="x")
        nc.sync.dma_start(xtile0[:], xT[active[0]])
        for qi, (lo, hi) in enumerate(W_QUARTERS):
            for j in range(KP):
                if qi == 0 and j == 0:
                    continue
                nc.sync.dma_start(w_sb[:, 2 * j:2 * j + 2, lo:hi],
                                  wT[:, 2 * j:2 * j + 2, lo:hi])

        mask_sb = const.tile([P, N_TI], f32)
        nc.gpsimd.dma_start(mask_sb[:], mask[:])
        tv = fin.tile([P, N_TI], i32)
        nc.gpsimd.dma_start(tv[:], tgtT[:])

        # sf: cols 0:64 = masked dots, cols 64:128 = per-token local sum-exp
        sf = spool.tile([P, 2 * N_TI], f32)
        D_sb = spool.tile([P, SLAB_TI], f32)  # per-token target dot (slab)
        for ti in skip_tiles:
            nc.vector.memset(sf[:, SCOL + ti:SCOL + ti + 1], 1.0)

        # fin-phase tiles touched from inside the loop
        gl = fin.tile([P, 2 * N_TI], f32)
        validf = fin.tile([P, N_TI], f32)
        stats = fin.tile([P, 4], f32)   # [tl_b0, tl_b1(part), nv_b0, nv_b1]
        t1b = fin.tile([P, 1], f32)
        lse = fin.tile([P, N_TI], f32)
        tl = fin.tile([P, N_TI], f32)
        vT = fin.tile([2, P], f32)
        LT = fin.tile([2, 1], f32)
        sqT = fin.tile([2, 1], f32)
        rT = fin.tile([2, 1], f32)
        nbT = fin.tile([2, 1], f32)
        r2 = fin.tile([1, 2], f32)
        nb2 = fin.tile([1, 2], f32)
        nbs = fin.tile([1, 1], f32)
        inv = fin.tile([1, 1], f32)
        scr = fin.tile([1, 1], f32)

        # ---- main vocab-parallel logit pass, everything else hidden ----
        for idx, ti in enumerate(active):
            if idx == 0:
                x = xtile0
            else:
                x = xpool.tile([P, KT, P], f8, tag="x")
                nc.sync.dma_start(x[:], xT[ti])
            acc = apool.tile([P, 2], f32, tag="acc")
            for g, blocks in enumerate(GROUPS):
                pt = psum.tile([P, 2048], f32, tag="pt")
                for off, wd, voff in blocks:
                    for j in range(KP):
                        nc.tensor.matmul(
                            pt[:, off:off + wd], x[:, 2 * j:2 * j + 2, :],
                            w_sb[:, 2 * j:2 * j + 2, voff:voff + wd],
                            start=(j == 0), stop=(j == KP - 1), perf_mode=DR)
                esc = epool.tile([P, 2048], bf16, tag="esc")
                gw = G_WIDTH[g]
                nc.scalar.activation(
                    esc[:, :gw], pt[:, :gw], Act.Exp, bias=zero_b[:],
                    scale=1.0 / W_SCALE, accum_out=acc[:, g:g + 1])
            nc.vector.tensor_tensor(out=sf[:, SCOL + ti:SCOL + ti + 1],
                                    in0=acc[:, 0:1], in1=acc[:, 1:2],
                                    op=Alu.add)

            # target-logit dots (DVE, bf16 in / f32 accum). Their DMAs are
            # issued from the Scalar queue so they trail the weight/x head
            # transfers (in-order engine => naturally gated late).
            if idx >= 4 and idx % 2 == 0 and (idx - 4) // 2 < SLAB_TI:
                si = (idx - 4) // 2
                a = dpool.tile([P, D], bf16, tag="da")
                b = dpool.tile([P, D], bf16, tag="db")
                nc.scalar.dma_start(a[:], x_slab[si * P:(si + 1) * P, :])
                nc.scalar.dma_start(b[:], w_tgt[si * P:(si + 1) * P, :])
                prod = dpool.tile([P, D], f32, tag="dp")
                # NOTE: fused tensor_tensor_reduce crashes the device on
                # this runtime path — separate mult + reduce instead.
                nc.vector.tensor_tensor(out=prod[:], in0=a[:], in1=b[:],
                                        op=Alu.mult)
                nc.vector.reduce_sum(out=D_sb[:, si:si + 1], in_=prod[:],
                                     axis=AX, op=Alu.add)
            if idx == 22:
                # place my slab's dots into my 8 columns, zero elsewhere
                for k in range(N_CORES):
                    nc.vector.tensor_tensor(
                        out=sf[:, DCOL + k * SLAB_TI:DCOL + (k + 1) * SLAB_TI],
                        in0=D_sb[:],
                        in1=mask_sb[:, k * SLAB_TI:(k + 1) * SLAB_TI],
                        op=Alu.mult)
            if idx == 26:
                # valid mask + counts + the whole sqrt-weight path: no
                # collective dependency, fully hidden under the loop
                nc.vector.tensor_scalar(
                    validf[:], tv[:], IGNORE, None, op0=Alu.not_equal)
                nc.vector.reduce_sum(out=stats[:, 2:3], in_=validf[:, :HB],
                                     axis=AX, op=Alu.add)
                nc.vector.reduce_sum(out=stats[:, 3:4], in_=validf[:, HB:],
                                     axis=AX, op=Alu.add)
                nc.sync.dma_start(vs_d[:], stats[:, 2:4])
                nc.sync.dma_start(vT[:], vs_d[:].rearrange("p f -> f p"))
                nc.vector.reduce_sum(out=LT[:], in_=vT[:], axis=AX,
                                     op=Alu.add)
            if idx == 30:
                nc.scalar.activation(sqT[:], LT[:], Act.Sqrt,
                                     bias=eps_b[:2, :])
                nc.vector.reciprocal(rT[:], sqT[:])
                nc.vector.tensor_tensor(out=nbT[:], in0=LT[:], in1=rT[:],
                                        op=Alu.mult)
                nc.sync.dma_start(r_d[:], rT[:])
                nc.sync.dma_start(nb_d[:], nbT[:])
                nc.sync.dma_start(r2[:], r_d[:].rearrange("p f -> f p"))
                nc.sync.dma_start(nb2[:], nb_d[:].rearrange("p f -> f p"))
                nc.vector.reduce_sum(out=nbs[:], in_=nb2[:], axis=AX,
                                     op=Alu.add)
                nc.vector.reciprocal(inv[:], nbs[:])
            if ti == ar1_trigger:
                # bulk reduction: dots + sum-exp of tiles 0..47, hidden
                # behind the remaining token tiles' compute
                nc.gpsimd.dma_start(ar1_in[:], sf[:, :AR1_W])
                nc.gpsimd.collective_compute(
                    "AllReduce", Alu.add, replica_groups=rg,
                    ins=[ar1_in[:]], outs=[ar1_out[:]])
                nc.gpsimd.dma_start(gl[:, :AR1_W], ar1_out[:])
            if idx == n_act - 8:
                # hidden: log/sub/mask/reduce for tiles 0..47 (AR#1 data)
                nc.scalar.activation(lse[:, :AR1_TI + 1],
                                     gl[:, SCOL:SCOL + AR1_TI + 1],
                                     Act.Ln, bias=zero_b[:])
                nc.vector.tensor_tensor(
                    out=tl[:, :AR1_TI + 1], in0=lse[:, :AR1_TI + 1],
                    in1=gl[:, DCOL:DCOL + AR1_TI + 1], op=Alu.subtract)
                nc.vector.tensor_tensor(
                    out=tl[:, :AR1_TI + 1], in0=tl[:, :AR1_TI + 1],
                    in1=validf[:, :AR1_TI + 1], op=Alu.mult)
                nc.vector.reduce_sum(out=stats[:, 0:1], in_=tl[:, :HB],
                                     axis=AX, op=Alu.add)
                nc.vector.reduce_sum(out=t1b[:], in_=tl[:, HB:AR1_TI + 1],
                                     axis=AX, op=Alu.add)
            if idx == n_act - 5:
                # second warm-up so the CC cores are hot for AR#2
                nc.gpsimd.collective_compute(
                    "AllReduce", Alu.add, replica_groups=rg,
                    ins=[wu2_in[:]], outs=[wu2_out[:]])

        # preload the ln table while AR#2 runs (the table pass puts the
        # load right before this dummy, whose deps are all constants)
        nc.scalar.activation(scr[:], ones[:1, :], Act.Ln, bias=zero_b[:1, :])

        nc.sync.dma_start(ar2_in[:], sf[:, AR1_W:])
        nc.gpsimd.collective_compute(
            "AllReduce", Alu.add, replica_groups=rg,
            ins=[ar2_in[:]], outs=[ar2_out[:]])
        nc.sync.dma_start(gl[:, AR1_W:], ar2_out[:])

        # ---- tail: only token tiles 48..63 + the final combine ----
        NT = N_TI - (AR1_TI + 1)  # 16
        nc.scalar.activation(lse[:, AR1_TI + 1:], gl[:, SCOL + AR1_TI + 1:],
                             Act.Ln, bias=zero_b[:])
        nc.vector.tensor_tensor(
            out=tl[:, AR1_TI + 1:], in0=lse[:, AR1_TI + 1:],
            in1=gl[:, DCOL + AR1_TI + 1:DCOL + N_TI], op=Alu.subtract)
        nc.vector.tensor_tensor(
            out=tl[:, AR1_TI + 1:], in0=tl[:, AR1_TI + 1:],
            in1=validf[:, AR1_TI + 1:], op=Alu.mult)
        t1c = fin.tile([P, 1], f32)
        nc.vector.reduce_sum(out=t1c[:], in_=tl[:, AR1_TI + 1:], axis=AX,
                             op=Alu.add)
        nc.vector.tensor_tensor(out=stats[:, 1:2], in0=t1b[:], in1=t1c[:],
                                op=Alu.add)

        pfin = psum.tile([1, 2], f32, tag="pt")
        nc.tensor.matmul(pfin[:], ones[:], stats[:, 0:2], start=True,
                         stop=True)
        tot = fin.tile([1, 2], f32)
        nc.vector.tensor_copy(out=tot[:], in_=pfin[:])
        la = fin.tile([1, 2], f32)
        nc.vector.tensor_tensor(out=la[:], in0=tot[:], in1=r2[:],
                                op=Alu.mult)
        las = fin.tile([1, 1], f32)
        nc.vector.reduce_sum(out=las[:], in_=la[:], axis=AX, op=Alu.add)
        res = fin.tile([1, 1], f32)
        nc.vector.tensor_tensor(out=res[:], in0=las[:], in1=inv[:],
                                op=Alu.mult)
        nc.sync.dma_start(loss_out[:], res[:])

    nc.compile()
    _CACHE[key] = nc
    return nc


def _skip_tiles(tgt):
    return frozenset(
        ti for ti in range(N_TI)
        if np.all(tgt[ti * P:(ti + 1) * P] == IGNORE))


def _prep_inputs(outputs, proj_weight, targets):
    import concourse.mybir as mybir
    f8np = mybir.dt.np(mybir.dt.float8e4)
    bf16np = ml_dtypes.bfloat16
    xf = np.ascontiguousarray(np.asarray(outputs, dtype=np.float32)
                              .reshape(T, D))
    w = np.asarray(proj_weight, dtype=np.float32)
    tgt = np.asarray(targets).astype(np.int32).reshape(T)

    # [N_TI, P(tok), KT, P(d)] -> [N_TI, P(d), KT, P(tok)]
    xT = np.ascontiguousarray(
        xf.reshape(N_TI, P, KT, P).transpose(0, 3, 2, 1)).astype(f8np)

    safe = np.where(tgt == IGNORE, 0, tgt)
    tgtT = np.ascontiguousarray(tgt.reshape(N_TI, P).T)

    in_maps = []
    for c in range(N_CORES):
        ws = w[c * V_SHARD:(c + 1) * V_SHARD]            # [4000, 1024]
        wTc = np.ascontiguousarray(
            (ws.T * W_SCALE).reshape(KT, P, V_SHARD)
            .transpose(1, 0, 2)).astype(f8np)
        sl = slice(c * T_SLAB, (c + 1) * T_SLAB)
        mk = np.zeros((P, N_TI), dtype=np.float32)
        mk[:, c * SLAB_TI:(c + 1) * SLAB_TI] = 1.0
        in_maps.append({
            "xT": xT,
            "wT": wTc,
            "w_tgt": np.ascontiguousarray(w[safe[sl]]).astype(bf16np),
            "x_slab": np.ascontiguousarray(xf[sl]).astype(bf16np),
            "tgtT": tgtT,
            "mask": mk,
        })
    return in_maps


def kernel(outputs, proj_weight, targets):
    from concourse.bass_utils import run_bass_kernel_spmd

    tgt = np.asarray(targets).astype(np.int32).reshape(T)
    nc = _build(_skip_tiles(tgt))
    in_maps = _prep_inputs(outputs, proj_weight, targets)
    res = run_bass_kernel_spmd(nc, in_maps, core_ids=list(range(N_CORES)))
    loss = np.asarray(res.results[0]["loss"], dtype=np.float32).reshape(())
    return loss
